# revision 3
# baseline (speedup 1.0000x reference)
"""Trainium2 Bass kernel for nn_BoundaryLoss (8-core data-parallel).

Math (see reference): loss = (1/C) * sum_c mean_{b,h,w} |pred_sdf_c - tgt_sdf_c|.

For any pred whose per-pixel logit spread is < 15, softmax probabilities are
never exactly 0.0 or 1.0 in f32, so both EDTs on the pred side saturate at
theta and pred_sdf == 0 identically (and no pred class-plane is empty).  The
host verifies that bound (np.max - np.min < 15 is sufficient) and falls back
to an exact slow path otherwise.  The device therefore only computes the
target-side SDFs:

  per image b, class c:
    d_plus_c  = min(5, dist to {target==c})      (interior pixels only)
    d_minus_c = min(5, dist to {target!=c}) = min_{c'!=c} d_plus_c'
    |tgt_sdf| = (d_plus + d_minus)/5             (one of the two is 0)

  Exact capped EDT on device (per 512x512 class mask, cap 25 = theta^2):
    pass 1 (x): two tensor_tensor_scan ops -> 1D distance s (capped 6), t = s^2
    pass 2 (y): exact min-plus via LSE in exp space on the TensorEngine:
        S[y] = sum_dy exp(B2-beta*dy^2) * exp(B1-beta*t[y+dy])   (banded matmul)
        D2   = trunc(-ln(S)/beta + (B1+B2)/beta + 0.70), capped at 25
      which is exact because D2 candidates are integers and the LSE
      overshoot ln(9)/beta + noise < 1 (trunc window).
    d = sqrt(D2) with fused per-partition row sums (ACT accum_out).

Per core: 2 images (batch shard).  Output: per-partition partial sums of
d_plus / d_minus per (image, class, y-block); host reduces, handles empty
classes, scales, and averages.
"""

import numpy as np

import concourse.bacc as bacc
import concourse.mybir as mybir
from concourse.mybir import AluOpType as Op
from concourse.tile import TileContext

P = 128
H = W = 512
YB = H // P          # 4 y-blocks
C = 4                # classes
BPC = 2              # images per core
NCORES = 8
B_TOTAL = BPC * NCORES

BETA = 5.0
B1 = 38.0            # exp bias on the data side
B2 = 4.0             # exp bias on the weight side
RND = 0.45           # rounding bias: makes trunc and RNE casts agree (overshoot <= 0.4423)
SCAP = 6.0           # 1D scan cap (>5 is enough; keeps s^2 bf16-exact)

F32 = mybir.dt.float32
BF16 = mybir.dt.bfloat16
I32 = mybir.dt.int32
I8 = mybir.dt.int8
Act = mybir.ActivationFunctionType


def _build_nc():
    nc = bacc.Bacc("TRN2", target_bir_lowering=False, debug=False)
    tgt_d = nc.dram_tensor("target", [BPC, H, W], I32, kind="ExternalInput")
    osp_d = nc.dram_tensor("osum_p", [P, BPC * C * YB], F32, kind="ExternalOutput")
    osm_d = nc.dram_tensor("osum_m", [P, BPC * C * YB], F32, kind="ExternalOutput")

    with TileContext(nc) as tc:
        with (
            tc.tile_pool(name="const", bufs=1) as cpool,
            tc.tile_pool(name="tgt", bufs=3) as tgt_pool,
            tc.tile_pool(name="mask", bufs=3) as m_pool,
            tc.tile_pool(name="fscan", bufs=3) as f_pool,
            tc.tile_pool(name="tsq", bufs=3) as t_pool,
            tc.tile_pool(name="emap", bufs=2 * YB + 1) as e_pool,
            tc.tile_pool(name="lnq", bufs=4) as u_pool,
            tc.tile_pool(name="d2r", bufs=4) as d2_pool,
            tc.tile_pool(name="dmap", bufs=2 * YB + 1) as d_pool,
            tc.tile_pool(name="scratch", bufs=2) as s_pool,
            tc.tile_pool(name="acc", bufs=1) as a_pool,
            tc.tile_pool(name="psum", bufs=4, space="PSUM") as psum_pool,
        ):
            # ---- constants ----
            ones = cpool.tile([P, W], BF16)
            nc.vector.memset(ones, 1.0)
            bias0 = cpool.tile([P, 1], F32)
            nc.vector.memset(bias0, 0.0)
            bias_b1 = cpool.tile([P, 1], F32)
            nc.vector.memset(bias_b1, B1)
            bias_b2 = cpool.tile([P, 1], F32)
            nc.vector.memset(bias_b2, B2)

            # Banded conv weights W[k, m] = exp(B2 - beta*(k - m - base)^2);
            # off-band entries underflow to 0 in bf16.
            wmats = []
            for name, base in (("wmain", 0), ("wprev", -P), ("wnext", P)):
                idx = cpool.tile([P, P], I32, tag=f"idx_{name}")
                nc.gpsimd.iota(idx, [[-1, P]], base=base, channel_multiplier=1)
                sq = cpool.tile([P, P], F32, tag=f"sq_{name}")
                nc.scalar.activation(sq, idx, Act.Square, bias=bias0[:])
                wm = cpool.tile([P, P], BF16, tag=name)
                nc.scalar.activation(wm, sq, Act.Exp, bias=bias_b2[:], scale=-BETA)
                wmats.append(wm)
            wmain, wprev, wnext = wmats

            accp = a_pool.tile([P, BPC * C * YB], F32)
            accm = a_pool.tile([P, BPC * C * YB], F32)

            for b in range(BPC):
                # ---- pass 1: masks + x-direction 1D distance, squared ----
                e_tiles = []
                for yb in range(YB):
                    tgt = tgt_pool.tile([P, W], I32)
                    nc.sync.dma_start(tgt, tgt_d[b, yb * P:(yb + 1) * P, :])
                    mw = m_pool.tile([P, C, W], BF16)
                    for c in range(C):
                        # (t != c) * SCAP  ->  {0 inside class, SCAP outside}
                        nc.vector.tensor_scalar(
                            mw[:, c], tgt, c, SCAP, Op.not_equal, Op.mult
                        )
                    fw = f_pool.tile([P, C, W], BF16)
                    for c in range(C):
                        nc.vector.tensor_tensor_scan(
                            fw[:, c], ones, mw[:, c], SCAP, Op.add, Op.min
                        )
                    ts = t_pool.tile([P, C, W], BF16)
                    for c in range(C):
                        nc.vector.tensor_tensor_scan(
                            ts[:, c][:, ::-1], ones, fw[:, c][:, ::-1],
                            SCAP, Op.add, Op.min,
                        )
                    # t = s^2 ; e = exp(B1 - beta * t)   (wide ops)
                    nc.scalar.activation(ts[:], ts[:], Act.Square, bias=bias0[:])
                    ew = e_pool.tile([P, C, W], BF16)
                    nc.scalar.activation(
                        ew[:], ts[:], Act.Exp, bias=bias_b1[:], scale=-BETA
                    )
                    e_tiles.append(ew)

                # ---- pass 2: y-direction exact min-plus via exp-space matmul ----
                d_tiles = []
                for yb in range(YB):
                    dd = d_pool.tile([P, C, W], BF16)
                    for c in range(C):
                        ps = psum_pool.tile([P, W], F32)
                        ops = [(wmain, yb)]
                        if yb > 0:
                            ops.append((wprev, yb - 1))
                        if yb < YB - 1:
                            ops.append((wnext, yb + 1))
                        for i, (wm, src) in enumerate(ops):
                            nc.tensor.matmul(
                                ps, wm, e_tiles[src][:, c],
                                start=(i == 0), stop=(i == len(ops) - 1),
                            )
                        uu = u_pool.tile([P, W], F32)
                        nc.scalar.activation(uu, ps, Act.Ln, bias=bias0[:])
                        qq = u_pool.tile([P, W], F32, tag="qq")
                        nc.vector.tensor_scalar(
                            qq, uu, -1.0 / BETA, (B1 + B2) / BETA + RND,
                            Op.mult, Op.add,
                        )
                        d2r = d2_pool.tile([P, W], I8)
                        nc.vector.tensor_scalar(d2r, qq, 25.49, None, Op.min)
                        # d = sqrt(D2), fused row-sum of d_plus
                        col = (b * C + c) * YB + yb
                        nc.scalar.activation(
                            dd[:, c], d2r, Act.Sqrt, bias=bias0[:],
                            accum_out=accp[:, col:col + 1],
                        )
                    d_tiles.append(dd)

                # ---- d_minus = min of the other classes' d_plus, row-summed ----
                for yb in range(YB):
                    dd = d_tiles[yb]
                    m01 = s_pool.tile([P, W], BF16, tag="m01")
                    m23 = s_pool.tile([P, W], BF16, tag="m23")
                    nc.vector.tensor_tensor(m01, dd[:, 0], dd[:, 1], Op.min)
                    nc.vector.tensor_tensor(m23, dd[:, 2], dd[:, 3], Op.min)
                    pair = {0: (dd[:, 1], m23), 1: (dd[:, 0], m23),
                            2: (dd[:, 3], m01), 3: (dd[:, 2], m01)}
                    for c in range(C):
                        col = (b * C + c) * YB + yb
                        dm = s_pool.tile([P, W], BF16, tag="dm")
                        a, m = pair[c]
                        nc.vector.scalar_tensor_tensor(
                            dm, a, 0.0, m, Op.add, Op.min,
                            accum_out=accm[:, col:col + 1],
                        )

            nc.sync.dma_start(osp_d[:, :], accp[:])
            nc.sync.dma_start(osm_d[:, :], accm[:])

    nc.compile()
    return nc


_NC = None


def _get_nc():
    global _NC
    if _NC is None:
        _NC = _build_nc()
    return _NC


def _exact_fallback(pred, target):
    """Exact numpy implementation of the reference (slow; adversarial inputs only)."""
    THETA0, THETA, R = 3.0, 5.0, 5
    offs = [(dy, dx, float(np.hypot(dy, dx)))
            for dy in range(-R, R + 1) for dx in range(-R, R + 1)
            if np.hypot(dy, dx) <= THETA]

    def capped_edt(ts):
        B, Hh, Ww = ts.shape
        pad = np.zeros((B, Hh + 2 * R, Ww + 2 * R), bool)
        pad[:, R:-R, R:-R] = ts
        d = np.full((B, Hh, Ww), THETA, np.float32)
        for dy, dx, dist in offs:
            win = pad[:, R + dy:R + dy + Hh, R + dx:R + dx + Ww]
            d = np.minimum(d, np.where(win, np.float32(dist), np.float32(THETA)))
        return d

    def compute_sdf(mask):
        sdf_pos = capped_edt(mask == 1.0)
        sdf_neg = capped_edt(mask == 0.0)
        sdf = np.clip(sdf_pos - sdf_neg, -THETA, THETA) / THETA
        empty = mask.sum(axis=(1, 2)) == 0.0
        return np.where(empty[:, None, None], np.float32(THETA0), sdf).astype(np.float32)

    x = pred.astype(np.float32)
    x = x - x.max(axis=1, keepdims=True)
    ex = np.exp(x)
    p = ex / ex.sum(axis=1, keepdims=True)
    Cn = pred.shape[1]
    loss = np.float32(0.0)
    for c in range(Cn):
        ps = compute_sdf(p[:, c].astype(np.float32))
        ts = compute_sdf((target == c).astype(np.float32))
        loss += np.abs(ps - ts).mean(dtype=np.float32)
    return np.float32(loss / Cn)


def kernel(pred: np.ndarray, target: np.ndarray) -> np.ndarray:
    pred = np.asarray(pred)
    target = np.asarray(target)

    # Soundness guards for the pred_sdf == 0 shortcut (never trip on randn /
    # randint inputs; exact slow path otherwise).
    gap_ok = float(pred.max()) - float(pred.min()) < 15.0
    tgt_ok = bool(((target >= 0) & (target < C)).all())
    if not (gap_ok and tgt_ok):
        return _exact_fallback(pred, target)

    from concourse.bass_utils import run_bass_kernel_spmd

    nc = _get_nc()
    in_maps = [
        {"target": np.ascontiguousarray(target[i * BPC:(i + 1) * BPC])}
        for i in range(NCORES)
    ]
    res = run_bass_kernel_spmd(nc, in_maps, list(range(NCORES))).results

    # host reduction
    npx = H * W
    total = 0.0
    present = np.array([
        [(target[b] == c).any() for c in range(C)] for b in range(B_TOTAL)
    ])
    for core in range(NCORES):
        sp = res[core]["osum_p"].astype(np.float64)  # [P, BPC*C*YB]
        sm = res[core]["osum_m"].astype(np.float64)
        per_col = sp.sum(axis=0) + sm.sum(axis=0)
        per_bc = per_col.reshape(BPC * C, YB).sum(axis=1)  # sum over y-blocks
        for bb in range(BPC):
            b = core * BPC + bb
            for c in range(C):
                if present[b, c]:
                    total += per_bc[bb * C + c] / (5.0 * npx)
                else:
                    total += 3.0 * npx / npx  # reference: |0 - theta0| per pixel
    loss = total / (B_TOTAL * C)
    return np.float32(loss)
